# revision 26
# baseline (speedup 1.0000x reference)
"""EntityAttention Trainium2 kernel (nn_EntityAttention_31525059952740), v3.

Per batch b -> core b (16 entities, 64 events):
  scoresT[s,(h,e)] = toksT.T @ wtil       (wtil = (q*scale) @ Wk folded on host)
  E = exp(scoresT) bf16;  denom = masksT.T @ E  (PE);  srec = 1/denom
  attnT = E * maskT (per-entity partition scalar)
  V = toks @ WvT;  po[d,(4e,64)] = V.T @ attnT   (unnormalized PV)
  outT_u = copy(po); outT = outT_u * srec_bc     (normalize once the DRAM
           round-trip broadcast of srec lands; decouples PSUM drain)
  O = outT.T @ WoT -> bf16 rows; host adds bo + bv@Wo.T and gathers.

Cost-model-driven layout: bf16 matmuls (f32r pays 4x under 256 free rows),
contiguous >=512B DMA runs, PE warm-up matmuls against the p-state ramp,
PSUM = 8 banks: warm/scores/pS ring(1) + pv/pO(4) + po dc-pairs(3).
Engine queues are emitted in consumption order; po copies and recip are
kept OFF the mask-heavy queues so the po ring never backs up PV.
"""

import numpy as np

import concourse.bass as bass
import concourse.tile as tile
import concourse.mybir as mybir
from concourse import bacc
from concourse.bass_utils import run_bass_kernel_spmd

NB, SL, NH, EN, NE, HEADS = 8, 512, 512, 16, 64, 2
DH = NH // HEADS          # 256
P = 128
NCHUNK = NH // P          # 4 hidden-dim chunks
SCHUNK = SL // P          # 4 sequence chunks
HE = HEADS * NE           # 128
SCALE = 1.0 / np.sqrt(DH).astype(np.float32)
N_WARM = 12

F32 = mybir.dt.float32
BF16 = mybir.dt.bfloat16
NPBF16 = mybir.dt.np(BF16)

_CACHE = {}


def _build():
    nc = bacc.Bacc("TRN2", target_bir_lowering=False, debug=False, num_devices=NB)

    # ---- I/O (host-prepacked; every DMA contiguous) ----
    sk_d = [nc.dram_tensor(f"s{k}", [P, SL + HE + NH], BF16,
                           kind="ExternalInput").ap() for k in (0, 1, 2, 3)]
    # masks: cols 0:64 f32 per-(sc,ent) 0/1 scalars; cols 64:96 the same
    # values as bf16 (bitcast view) for the denominator matmul.
    masks_d = nc.dram_tensor("masks", [P, 96], F32, kind="ExternalInput").ap()
    wo_d = nc.dram_tensor("wo", [P, NCHUNK * NH], BF16,
                          kind="ExternalInput").ap()
    out_d = nc.dram_tensor("out", [EN * NE, NH], BF16, kind="ExternalOutput").ap()

    EXP = mybir.ActivationFunctionType.Exp
    CPY = mybir.ActivationFunctionType.Copy

    with tile.TileContext(nc) as tc:
        with (
            tc.tile_pool(name="sb", bufs=1) as sb,
            tc.tile_pool(name="ps", bufs=1, space="PSUM") as ps,
            tc.tile_pool(name="dram", bufs=1, space="DRAM") as dram,
        ):
            # ---------- input DMAs (sync queue, consumption order) --------
            sk_t = []
            for k in range(4):
                t = sb.tile([P, SL + HE + NH], BF16, tag=f"sk{k}",
                            name=f"sk{k}")
                nc.sync.dma_start(t[:], sk_d[k])
                sk_t.append(t)
            masks_t = sb.tile([P, 96], F32, tag="masks")
            nc.sync.dma_start(masks_t[:], masks_d)
            wo_t = sb.tile([P, NCHUNK, NH], BF16, tag="wo")
            nc.sync.dma_start(
                wo_t[:], wo_d.rearrange("p (c d) -> p c d", c=NCHUNK))

            masksF = masks_t[:, 0:64].rearrange("p (c e) -> p c e", c=SCHUNK)
            masksB = masks_t[:, 64:96].bitcast(BF16).rearrange(
                "p (c e) -> p c e", c=SCHUNK)

            def toks(hc):
                return sk_t[hc][:, 0:SL]

            def wtil(hc):
                return sk_t[hc][:, SL:SL + HE]

            def wvc(hc):
                return sk_t[hc][:, SL + HE:]

            # ---------- PE warm-up (p-state ramp) -------------------------
            scratch = sb.tile([P, 256], BF16, tag="scratch")
            nc.gpsimd.memset(scratch[:], 0)
            warm_ps = ps.tile([P, 256], F32, tag="bank", bufs=8, name="warm")
            for _ in range(N_WARM):
                nc.tensor.matmul(warm_ps[:], scratch[:, 0:128], scratch[:],
                                 start=True, stop=True)

            # ---------- scores + V, streamed per hidden chunk -------------
            # one accumulation group per PSUM bank (HW constraint)
            pss = [ps.tile([P, HE], F32, tag="bank", bufs=8, name=f"pss{sc}")
                   for sc in range(SCHUNK)]
            pv = [ps.tile([P, NH], F32, tag="bank", bufs=8, name=f"pv{i}")
                  for i in range(SCHUNK)]
            for hc in range(NCHUNK):
                for sc in range(SCHUNK):
                    nc.tensor.matmul(
                        pss[sc][:],
                        toks(hc)[:, sc * P:(sc + 1) * P], wtil(hc),
                        start=(hc == 0), stop=(hc == NCHUNK - 1))
                for i in range(SCHUNK):
                    nc.tensor.matmul(
                        pv[i][:], toks(hc)[:, i * P:(i + 1) * P], wvc(hc),
                        start=(hc == 0), stop=(hc == NCHUNK - 1))

            # ---------- exp (2 ops), V copies, masking --------------------
            e_t = sb.tile([P, SCHUNK, HE], BF16, tag="E")
            for sc in range(SCHUNK):
                nc.scalar.activation(e_t[:, sc, :], pss[sc][:], EXP)

            v = [sb.tile([P, NH], BF16, tag=f"v{i}", name=f"v{i}")
                 for i in range(SCHUNK)]
            H2 = NH // 2
            nc.vector.tensor_copy(v[0][:, :H2], pv[0][:, :H2])  # DVE: v0a
            attnT = {}
            for g in range(4):
                for sc in range(SCHUNK):
                    attnT[(g, sc)] = sb.tile([P, 4, HE], BF16,
                                             tag=f"attnT{g}_{sc}",
                                             name=f"attnT{g}_{sc}")

            def mask_op(eng, g, sc, k):
                col = 4 * g + k
                if eng is nc.scalar:
                    eng.activation(attnT[(g, sc)][:, k, :], e_t[:, sc, :],
                                   CPY, scale=masksF[:, sc, col:col + 1])
                else:
                    eng.tensor_scalar_mul(
                        attnT[(g, sc)][:, k, :], e_t[:, sc, :],
                        masksF[:, sc, col:col + 1])

            # Act: first halves feed PV dc01 blocks
            nc.scalar.activation(v[1][:, :H2], pv[1][:, :H2], CPY)
            nc.scalar.activation(v[2][:, :H2], pv[2][:, :H2], CPY)
            nc.scalar.activation(v[3][:, :H2], pv[3][:, :H2], CPY)

            # ---------- denominators (PE) ---------------------------------
            pS = ps.tile([EN, HE], F32, tag="bank", bufs=8, name="pS")
            for sc in range(SCHUNK):
                nc.tensor.matmul(pS[:], masksB[:, sc, :], e_t[:, sc, :],
                                 start=(sc == 0), stop=(sc == SCHUNK - 1))

            # DVE: g0 masks k0-k2 (Pool does every k3); recip after sc1
            for sc in range(SCHUNK):
                for k in range(3):
                    mask_op(nc.vector, 0, sc, k)
                mask_op(nc.gpsimd, 0, sc, 3)
                if sc == 1:
                    srec = sb.tile([EN, HE], BF16, tag="srec")
                    with nc.allow_low_precision(
                            reason="bf16 softmax denominators"):
                        nc.vector.reciprocal(srec[:], pS[:])
            srec_dram = dram.tile([EN, HE], BF16)
            nc.sync.dma_start(srec_dram[:], srec[:])
            srec_bc = []
            for g in range(4):
                t = sb.tile([P, 4, HE], BF16, tag=f"srecbc{g}",
                            name=f"srecbc{g}")
                sd = srec_dram[g * 4:(g + 1) * 4, :]
                nc.sync.dma_start(
                    t[:], bass.AP(tensor=sd.tensor, offset=sd.offset,
                                  ap=[[0, P], *sd.ap]))
                srec_bc.append(t)

            # rest of the masks: DVE k0-k2, Pool k3; v-b halves slotted in
            nc.vector.tensor_copy(v[0][:, H2:], pv[0][:, H2:])
            nc.vector.tensor_copy(v[1][:, H2:], pv[1][:, H2:])
            for g in (1, 2, 3):
                for sc in range(SCHUNK):
                    for k in range(3):
                        mask_op(nc.vector, g, sc, k)
                    mask_op(nc.gpsimd, g, sc, 3)
                if g == 1:
                    nc.vector.tensor_copy(v[2][:, H2:], pv[2][:, H2:])
                    nc.vector.tensor_copy(v[3][:, H2:], pv[3][:, H2:])

            # ---------- PV: po = V.T @ attnT, dc-pairs share a bank -------
            po = {}

            def pv_block(g, half):
                key = (g, half)
                po[key] = ps.tile([P, 2, 4 * NE], F32, tag="bank", bufs=8,
                                  name=f"po{g}_{half}")
                for j in range(2):
                    dc = 2 * half + j
                    for sc in range(SCHUNK):
                        nc.tensor.matmul(
                            po[key][:, j, :],
                            v[sc][:, dc * P:(dc + 1) * P],
                            attnT[(g, sc)][:, :, half * NE:(half + 1) * NE],
                            start=(sc == 0), stop=(sc == SCHUNK - 1))

            for g in range(4):
                pv_block(g, 0)
                pv_block(g, 1)

            # Act: g3 k0/k1 masks sandwiched between its po copies
            outT_u = [sb.tile([P, NCHUNK, 4, NE], BF16, tag=f"outTu{g}",
                              name=f"outTu{g}") for g in range(4)]
            outT = [sb.tile([P, NCHUNK, 4, NE], BF16, tag=f"outT{g}",
                            name=f"outT{g}") for g in range(4)]

            def po_copy(eng, g, half):
                dst = outT_u[g][:, 2 * half:2 * half + 2, :, :]
                if eng is nc.scalar:
                    eng.activation(dst, po[(g, half)][:], CPY)
                else:
                    eng.tensor_copy(dst, po[(g, half)][:])

            for g in range(2):
                po_copy(nc.scalar, g, 0)
                po_copy(nc.scalar, g, 1)

            # ---------- normalize ----------------------------------------
            def srec_bc_ap(g, half):
                sl = srec_bc[g][:, :, half * NE:(half + 1) * NE]
                return bass.AP(tensor=sl.tensor, offset=sl.offset,
                               ap=[sl.ap[0], [0, 2], *sl.ap[1:]])

            for g in range(2):
                for half in range(2):
                    nc.vector.tensor_mul(
                        outT[g][:, 2 * half:2 * half + 2, :, :],
                        outT_u[g][:, 2 * half:2 * half + 2, :, :],
                        srec_bc_ap(g, half))
            # g2/g3: srec has landed -> fused copy+normalize straight from PSUM
            for g in (2, 3):
                for half in range(2):
                    nc.vector.tensor_mul(
                        outT[g][:, 2 * half:2 * half + 2, :, :],
                        po[(g, half)][:], srec_bc_ap(g, half))

            # ---------- O projection + output -----------------------------
            for g in range(4):
                for lp in range(2):
                    pair = g * 2 + lp
                    pO = ps.tile([P, NH], F32, tag="bank", bufs=8,
                                 name=f"pO{pair}")
                    for hc in range(NCHUNK):
                        nc.tensor.matmul(
                            pO[:], outT[g][:, hc, 2 * lp:2 * lp + 2, :],
                            wo_t[:, hc, :],
                            start=(hc == 0), stop=(hc == NCHUNK - 1))
                    o_sb = sb.tile([P, NH], BF16, tag=f"osb{pair}",
                                   name=f"osb{pair}")
                    if pair % 2 == 0 or pair == 7:
                        nc.scalar.activation(o_sb[:], pO[:], CPY)
                    else:
                        nc.vector.tensor_copy(o_sb[:], pO[:])
                    nc.sync.dma_start(out_d[pair * P:(pair + 1) * P, :],
                                      o_sb[:])

    nc.compile()
    return nc


def _get_nc():
    if "nc" not in _CACHE:
        _CACHE["nc"] = _build()
    return _CACHE["nc"]


def _fast_run(nc, in_maps):
    """Repeat-call path: cached jitted shard_map over the bass PJRT primitive."""
    import jax
    from jax.sharding import Mesh, PartitionSpec
    from jax.experimental.shard_map import shard_map
    import concourse.mybir as mybir_
    from concourse import bass2jax

    if "runner" not in _CACHE:
        bass2jax.install_neuronx_cc_hook()
        part_name = (nc.partition_id_tensor.name
                     if nc.partition_id_tensor else None)
        in_names, out_names, out_avals = [], [], []
        for alloc in nc.m.functions[0].allocations:
            if not isinstance(alloc, mybir_.MemoryLocationSet):
                continue
            name = alloc.memorylocations[0].name
            if alloc.kind == "ExternalInput":
                if name != part_name:
                    in_names.append(name)
            elif alloc.kind == "ExternalOutput":
                out_names.append(name)
                out_avals.append(jax.core.ShapedArray(
                    tuple(alloc.tensor_shape), mybir_.dt.np(alloc.dtype)))
        n_params = len(in_names)
        all_in_names = in_names + out_names
        if part_name is not None:
            all_in_names = all_in_names + [part_name]

        def _body(*args):
            operands = list(args)
            if part_name is not None:
                operands.append(bass2jax.partition_id_tensor())
            outs = bass2jax._bass_exec_p.bind(
                *operands,
                out_avals=tuple(out_avals),
                in_names=tuple(all_in_names),
                out_names=tuple(out_names),
                lowering_input_output_aliases=(),
                sim_require_finite=True,
                sim_require_nnan=True,
                nc=nc,
            )
            return tuple(outs)

        devices = jax.devices()[:NB]
        mesh = Mesh(np.asarray(devices), ("core",))
        n_outs = len(out_names)
        sharded = jax.jit(
            shard_map(_body, mesh=mesh,
                      in_specs=(PartitionSpec("core"),) * (n_params + n_outs),
                      out_specs=(PartitionSpec("core"),) * n_outs,
                      check_rep=False),
            donate_argnums=tuple(range(n_params, n_params + n_outs)),
            keep_unused=True,
        )
        _CACHE["runner"] = (sharded, in_names, out_names, out_avals)

    sharded, in_names, out_names, out_avals = _CACHE["runner"]
    concat_in = [
        np.concatenate([np.asarray(m[name]) for m in in_maps], axis=0)
        for name in in_names
    ]
    concat_zeros = [
        np.zeros((NB * av.shape[0], *av.shape[1:]), av.dtype)
        for av in out_avals
    ]
    out_arrs = sharded(*concat_in, *concat_zeros)
    return [
        {name: np.asarray(out_arrs[i]).reshape(NB, *out_avals[i].shape)[c]
         for i, name in enumerate(out_names)}
        for c in range(NB)
    ]


def kernel(tokens_embed, entities, events_embed, entity_num, entity_masks,
           select_event, Wq, Wk, Wv, bq, bk, bv, Wo, bo):
    tokens_embed = np.asarray(tokens_embed, dtype=np.float32)
    entities = np.asarray(entities)
    events_embed = np.asarray(events_embed, dtype=np.float32)
    entity_masks = np.asarray(entity_masks)
    select_event = np.asarray(select_event)
    Wq = np.asarray(Wq, dtype=np.float32)
    Wk = np.asarray(Wk, dtype=np.float32)
    Wv = np.asarray(Wv, dtype=np.float32)
    Wo = np.asarray(Wo, dtype=np.float32)
    bq = np.asarray(bq, dtype=np.float32)
    bk = np.asarray(bk, dtype=np.float32)
    bv = np.asarray(bv, dtype=np.float32)
    bo = np.asarray(bo, dtype=np.float32)

    nc = _get_nc()

    q_s = (events_embed @ Wq.T + bq) * SCALE          # [NE, NH]
    # fold K projection into the query side (bk cancels in softmax):
    wtil = np.empty((NH, HE), dtype=np.float32)
    for h in range(HEADS):
        hs = slice(h * DH, (h + 1) * DH)
        wtil[:, h * NE:(h + 1) * NE] = (q_s[:, hs] @ Wk[hs, :]).T
    wtil_r = wtil.reshape(NCHUNK, P, HE)
    wv_r = np.ascontiguousarray(Wv.T).reshape(NCHUNK, P, NH)
    wo_r = np.ascontiguousarray(Wo.T).reshape(NCHUNK, P, NH)
    wo_pc = np.ascontiguousarray(
        wo_r.transpose(1, 0, 2).reshape(P, NCHUNK * NH)).astype(NPBF16)
    # attn rows sum to 1 -> bv contributes bv @ Wo.T; applied host-side.
    bo2 = (bo + bv @ Wo.T).astype(np.float32)

    shared = {"wo": wo_pc}
    in_maps = []
    for c in range(NB):
        toks_r = np.ascontiguousarray(tokens_embed[c].T).reshape(NCHUNK, P, SL)
        streams = {}
        for k in (0, 1, 2, 3):
            sk = np.concatenate([toks_r[k], wtil_r[k], wv_r[k]],
                                axis=1).astype(NPBF16)
            streams[f"s{k}"] = np.ascontiguousarray(sk)
        # masks[p, sc*16+ent] = entities[c, ent, sc*128 + p] (f32 + bf16 view)
        m = entities[c].astype(np.float32)            # [EN, SL]
        mT = np.ascontiguousarray(
            m.reshape(EN, SCHUNK, P).transpose(2, 1, 0).reshape(P, 64))
        mpack = np.zeros((P, 96), dtype=np.float32)
        mpack[:, 0:64] = mT
        mpack[:, 64:96] = np.ascontiguousarray(
            mT.astype(NPBF16)).view(np.float32)
        streams["masks"] = mpack
        in_maps.append({**streams, **shared})

    if "ran_once" not in _CACHE:
        res = run_bass_kernel_spmd(nc, in_maps, core_ids=list(range(NB)))
        results = res.results
        _CACHE["ran_once"] = True
    else:
        results = _fast_run(nc, in_maps)
    full = np.concatenate(
        [results[c]["out"].astype(np.float32) for c in range(NB)], axis=0)
    full += bo2[None, :]

    # ragged selection (mirrors the reference indexing)
    assert int(entity_num) == EN
    entity_index = np.flatnonzero(entity_masks.reshape(-1))
    pair_sel = (select_event[:, None, :] & entity_masks[:, :, None])
    pair_sel = pair_sel.reshape(-1, NE)[entity_index].reshape(-1)
    event_entity_index = np.flatnonzero(pair_sel)

    sel_rows = (entity_index[:, None] * NE + np.arange(NE)[None, :]).reshape(-1)
    return full[sel_rows][event_entity_index]
